# revision 10
# baseline (speedup 1.0000x reference)
"""Trainium2 Bass kernel for HDGradientCompressionLayer forward.

Reference computation: y = einsum("bsd,df->bsf", x, W) + b
  x: (4, 4096, 1024) f32, W: (1024, 1024) f32, b: (1024,) f32.

Strategy (data-parallel across 8 cores, per sharding hint):
  Flatten x to (16384, 1024); each core gets 2048 rows. Per core the
  kernel computes y_shard = x_shard @ W + b:
    - x rowblocks and W k-blocks are cast-loaded f32->bf16 on the
      SWDGE queue, interleaved so x0-x2 and W land early,
    - per rowblock the PE transposes the 8 [128,128] x tiles into PSUM
      (~0.6us burst) and scalar/vector alternate evicting them to SBUF,
    - rowblocks 0-2 run k-outer across 6 PSUM banks so the PE can
      follow W's k-block-by-k-block arrival (this phase also absorbs
      the PE clock ramp; warmup matmuls fill the leading gaps),
    - rowblocks 3-15 then stream 16 bf16 matmuls each (N=512,
      PSUM-accumulated over the 8 d-blocks) at full PE rate,
    - DVE adds the (partition-broadcast) f32 bias during PSUM->SBUF
      eviction, scalar HWDGE stores f32 y rowblocks (4KB descriptors).
"""

import os
from contextlib import ExitStack

import numpy as np

import concourse.bass as bass
import concourse.bacc as bacc
import concourse.tile as tile
from concourse import mybir
from concourse.bass_utils import run_bass_kernel_spmd
from concourse.masks import make_identity

N_CORES = 8
B, S, D = 4, 4096, 1024
F = 1024
ROWS_TOTAL = B * S          # 16384
ROWS = ROWS_TOTAL // N_CORES  # 2048 per core
P = 128
NSPLIT = 512                # one PSUM bank of f32
KB = D // P                 # 8 contraction blocks
NB = F // NSPLIT            # 2 psum banks per rowblock
GROUP = 3                   # rowblocks in the k-outer warm phase


def build_nc(rows: int = ROWS) -> bass.Bass:
    nc = bacc.Bacc("TRN2", target_bir_lowering=False, debug=False)
    x = nc.dram_tensor("x", [rows, D], mybir.dt.float32, kind="ExternalInput").ap()
    W = nc.dram_tensor("W", [D, F], mybir.dt.float32, kind="ExternalInput").ap()
    b = nc.dram_tensor("b", [F], mybir.dt.float32, kind="ExternalInput").ap()
    y = nc.dram_tensor("y", [rows, F], mybir.dt.float32, kind="ExternalOutput").ap()

    RB = rows // P     # rowblocks

    with tile.TileContext(nc) as tc, ExitStack() as ctx:
        const = ctx.enter_context(tc.tile_pool(name="const", bufs=1))
        xp = ctx.enter_context(tc.tile_pool(name="xp", bufs=RB))
        xfp = ctx.enter_context(tc.tile_pool(name="xfp", bufs=4))
        xtp = ctx.enter_context(tc.tile_pool(name="xtp", bufs=RB))
        yp = ctx.enter_context(tc.tile_pool(name="yp", bufs=4))
        psp = ctx.enter_context(tc.tile_pool(name="psp", bufs=1, space="PSUM"))

        # W cast to bf16, laid out [p, k, f] with d = k*128 + p to match
        # the PE-transpose output layout of x.
        W_bf = const.tile([P, KB, F], mybir.dt.bfloat16)
        W_pkf = W.rearrange("(k p) f -> p k f", p=P)

        # Bias broadcast to all partitions, f32.
        b_bc = const.tile([P, F], mybir.dt.float32)

        # Identity for PE-based transposes; zeroed tile for clock warmup.
        ident = const.tile([P, P], mybir.dt.bfloat16)
        make_identity(nc, ident[:])
        warm = const.tile([P, P], mybir.dt.bfloat16)
        nc.any.memset(warm[:], 0.0)

        def ps0_tile():
            return psp.tile([P, NSPLIT], mybir.dt.float32, name="ps0", tag="ps0", bufs=GROUP)

        def ps1_tile():
            return psp.tile([P, NSPLIT], mybir.dt.float32, name="ps1", tag="ps1", bufs=GROUP)

        # x is fed through two DMA queues so it is not starved by the y
        # stores mid-kernel: even rowblocks cast-load on SWDGE, odd
        # rowblocks load f32 on the sync HWDGE queue and are cast to
        # bf16 by the scalar engine. W k-blocks interleave on SWDGE so
        # the k-outer phase can chase W's arrival.
        x_tiles = [None] * RB

        def load_x_even(rb: int):
            x_bf = xp.tile([P, D], mybir.dt.bfloat16, name="x_bf", tag="x_bf")
            nc.gpsimd.dma_start(x_bf[:], x[rb * P:(rb + 1) * P, :])  # cast load
            x_tiles[rb] = x_bf

        def load_x_odd(rb: int):
            x_f32 = xfp.tile([P, D], mybir.dt.float32, name="x_f32", tag="x_f32")
            nc.sync.dma_start(x_f32[:], x[rb * P:(rb + 1) * P, :])
            x_tiles[rb] = x_f32  # cast deferred to cast_x_odd

        def cast_x_odd(rb: int):
            x_bf = xp.tile([P, D], mybir.dt.bfloat16, name="x_bf", tag="x_bf")
            nc.scalar.copy(x_bf[:], x_tiles[rb][:])
            x_tiles[rb] = x_bf

        load_x_even(0)
        nc.gpsimd.dma_start(W_bf[:, 0, :], W_pkf[:, 0, :])
        load_x_even(2)
        nc.gpsimd.dma_start(W_bf[:, 1, :], W_pkf[:, 1, :])
        nc.gpsimd.dma_start(W_bf[:, 2, :], W_pkf[:, 2, :])
        nc.gpsimd.dma_start(b_bc[:], b.rearrange("(o f) -> o f", o=1).to_broadcast([P, F]))
        for k in range(3, KB):
            nc.gpsimd.dma_start(W_bf[:, k, :], W_pkf[:, k, :])
        for rb in range(4, RB, 2):
            load_x_even(rb)
        for rb in range(1, RB, 2):
            load_x_odd(rb)
        cast_x_odd(1)
        cast_x_odd(3)

        def warmup(n):
            for _ in range(n):
                nc.tensor.matmul(
                    warm_ps[:, 0:P], warm[:], warm[:, 0:1].to_broadcast([P, P]),
                    start=True, stop=True, skip_group_check=True,
                )

        def transpose(rb: int):
            # PE transposes the 8 k-tiles into one PSUM bank; DVE copies
            # them back to SBUF (scalar is busy with x casts + y issues).
            psT = psp.tile([P, KB, P], mybir.dt.bfloat16, name="psT", tag="psT", bufs=2)
            for k in range(KB):
                nc.tensor.transpose(psT[:, k, :], x_tiles[rb][:, k * P:(k + 1) * P], ident[:])
            xT = xtp.tile([P, KB, P], mybir.dt.bfloat16, name="xT", tag="xT")
            nc.vector.tensor_copy(xT[:], psT[:])
            return xT

        def evict(rb: int, pss) -> None:
            y_sb = yp.tile([P, F], mybir.dt.float32)
            for n in range(NB):
                nc.vector.tensor_add(
                    y_sb[:, n * NSPLIT:(n + 1) * NSPLIT],
                    pss[n][:],
                    b_bc[:, n * NSPLIT:(n + 1) * NSPLIT],
                )
            nc.scalar.dma_start(y[rb * P:(rb + 1) * P, :], y_sb[:])

        # PE warmup ramps the clock while the first x rowblock lands;
        # more warmups pad the gaps between the early transposes.
        warm_ps = ps0_tile()
        warmup(12)

        # Phase 1 - rowblocks 0..GROUP-1: transposes as x arrives, then
        # k-outer across 6 banks following W's k-block arrivals.
        xT_tiles = {}
        for r in range(GROUP):
            xT_tiles[r] = transpose(r)
            if r < GROUP - 1:
                warmup(4)
        psA = [(ps0_tile(), ps1_tile()) for _ in range(GROUP)]
        for k in range(KB):
            for r in range(GROUP):
                for n in range(NB):
                    nc.tensor.matmul(
                        psA[r][n][:],
                        xT_tiles[r][:, k, :],
                        W_bf[:, k, n * NSPLIT:(n + 1) * NSPLIT],
                        start=(k == 0),
                        stop=(k == KB - 1),
                    )
        for r in range(GROUP):
            evict(r, psA[r])

        # Phase 2 - rowblocks GROUP..RB-1 stream with k-inner; odd-x
        # casts are emitted two rowblocks ahead of their use.
        for rb in range(GROUP, RB):
            xT = transpose(rb)
            if rb + 2 < RB and (rb + 2) % 2 == 1:
                cast_x_odd(rb + 2)
            pss = (ps0_tile(), ps1_tile())
            for k in range(KB):
                for n in range(NB):
                    nc.tensor.matmul(
                        pss[n][:],
                        xT[:, k, :],
                        W_bf[:, k, n * NSPLIT:(n + 1) * NSPLIT],
                        start=(k == 0),
                        stop=(k == KB - 1),
                    )
            evict(rb, pss)

    nc.compile()
    return nc


_NC_CACHE: dict[int, bass.Bass] = {}


def _get_nc(rows: int = ROWS) -> bass.Bass:
    if rows not in _NC_CACHE:
        _NC_CACHE[rows] = build_nc(rows)
    return _NC_CACHE[rows]


def _run(in_maps, rows: int = ROWS, trace: bool = False):
    nc = _get_nc(rows)
    return run_bass_kernel_spmd(nc, in_maps, list(range(N_CORES)), trace=trace)


def kernel(x: np.ndarray, W: np.ndarray, b: np.ndarray) -> np.ndarray:
    x = np.ascontiguousarray(np.asarray(x, dtype=np.float32))
    W = np.ascontiguousarray(np.asarray(W, dtype=np.float32))
    b = np.ascontiguousarray(np.asarray(b, dtype=np.float32))
    x_flat = x.reshape(ROWS_TOTAL, D)
    in_maps = [
        {"x": np.ascontiguousarray(x_flat[c * ROWS:(c + 1) * ROWS]), "W": W, "b": b}
        for c in range(N_CORES)
    ]
    res = _run(in_maps, trace=bool(int(os.environ.get("BASS_KERNEL_TRACE", "0"))))
    y = np.concatenate([res.results[c]["y"] for c in range(N_CORES)], axis=0)
    return y.reshape(B, S, F)


# revision 11
# speedup vs baseline: 1.1033x; 1.1033x over previous
"""Trainium2 Bass kernel for HDGradientCompressionLayer forward.

Reference computation: y = einsum("bsd,df->bsf", x, W) + b
  x: (4, 4096, 1024) f32, W: (1024, 1024) f32, b: (1024,) f32.

Strategy (data-parallel across 8 cores, per sharding hint):
  Flatten x to (16384, 1024); each core gets 2048 rows. Per core the
  kernel computes y_shard = x_shard @ W + b:
    - rowblocks x0-x2 and all of W cast-load f32->bf16 on the SWDGE
      queue so W is fully resident by ~20us,
    - rowblocks x3-x15 load f32 on the sync HWDGE queue and are cast
      to bf16 by the scalar engine; small staging rings pace these
      loads behind consumption so they never starve W or the y stores,
    - per rowblock the PE transposes the 8 [128,128] x tiles into PSUM
      (~0.6us burst), DVE evicts them to SBUF,
    - rowblocks 0-2 run k-outer across 6 PSUM banks so the PE can
      follow W's k-block-by-k-block arrival (this phase also absorbs
      the PE clock ramp; warmup matmuls fill the leading gaps),
    - rowblocks 3-15 then stream 16 bf16 matmuls each (N=512,
      PSUM-accumulated over the 8 d-blocks) at full PE rate,
    - DVE adds the (partition-broadcast) f32 bias during PSUM->SBUF
      eviction, scalar HWDGE stores f32 y rowblocks (4KB descriptors).
"""

import os
from contextlib import ExitStack

import numpy as np

import concourse.bass as bass
import concourse.bacc as bacc
import concourse.tile as tile
from concourse import mybir
from concourse.bass_utils import run_bass_kernel_spmd
from concourse.masks import make_identity

N_CORES = 8
B, S, D = 4, 4096, 1024
F = 1024
ROWS_TOTAL = B * S          # 16384
ROWS = ROWS_TOTAL // N_CORES  # 2048 per core
P = 128
NSPLIT = 512                # one PSUM bank of f32
KB = D // P                 # 8 contraction blocks
NB = F // NSPLIT            # 2 psum banks per rowblock
GROUP = 3                   # rowblocks in the k-outer warm phase


def build_nc(rows: int = ROWS) -> bass.Bass:
    nc = bacc.Bacc("TRN2", target_bir_lowering=False, debug=False)
    x = nc.dram_tensor("x", [rows, D], mybir.dt.float32, kind="ExternalInput").ap()
    W = nc.dram_tensor("W", [D, F], mybir.dt.float32, kind="ExternalInput").ap()
    b = nc.dram_tensor("b", [F], mybir.dt.float32, kind="ExternalInput").ap()
    y = nc.dram_tensor("y", [rows, F], mybir.dt.float32, kind="ExternalOutput").ap()

    RB = rows // P     # rowblocks

    with tile.TileContext(nc) as tc, ExitStack() as ctx:
        const = ctx.enter_context(tc.tile_pool(name="const", bufs=1))
        xpe = ctx.enter_context(tc.tile_pool(name="xpe", bufs=GROUP))
        xpo = ctx.enter_context(tc.tile_pool(name="xpo", bufs=4))
        xfp = ctx.enter_context(tc.tile_pool(name="xfp", bufs=3))
        xtp = ctx.enter_context(tc.tile_pool(name="xtp", bufs=RB))
        yp = ctx.enter_context(tc.tile_pool(name="yp", bufs=4))
        psp = ctx.enter_context(tc.tile_pool(name="psp", bufs=1, space="PSUM"))

        # W cast to bf16, laid out [p, k, f] with d = k*128 + p to match
        # the PE-transpose output layout of x.
        W_bf = const.tile([P, KB, F], mybir.dt.bfloat16)
        W_pkf = W.rearrange("(k p) f -> p k f", p=P)

        # Bias broadcast to all partitions, f32.
        b_bc = const.tile([P, F], mybir.dt.float32)

        # Identity for PE-based transposes; zeroed tile for clock warmup.
        ident = const.tile([P, P], mybir.dt.bfloat16)
        make_identity(nc, ident[:])
        warm = const.tile([P, P], mybir.dt.bfloat16)
        nc.any.memset(warm[:], 0.0)

        def ps0_tile():
            return psp.tile([P, NSPLIT], mybir.dt.float32, name="ps0", tag="ps0", bufs=GROUP)

        def ps1_tile():
            return psp.tile([P, NSPLIT], mybir.dt.float32, name="ps1", tag="ps1", bufs=GROUP)

        x_tiles = [None] * RB

        # SWDGE: x0-x2 cast-loads interleaved with W k-blocks and bias.
        def load_x_front(rb: int):
            x_bf = xpe.tile([P, D], mybir.dt.bfloat16, name="x_bf", tag="x_bf")
            nc.gpsimd.dma_start(x_bf[:], x[rb * P:(rb + 1) * P, :])  # cast load
            x_tiles[rb] = x_bf

        load_x_front(0)
        nc.gpsimd.dma_start(W_bf[:, 0, :], W_pkf[:, 0, :])
        load_x_front(1)
        nc.gpsimd.dma_start(W_bf[:, 1, :], W_pkf[:, 1, :])
        load_x_front(2)
        nc.gpsimd.dma_start(W_bf[:, 2, :], W_pkf[:, 2, :])
        nc.gpsimd.dma_start(b_bc[:], b.rearrange("(o f) -> o f", o=1).to_broadcast([P, F]))
        for k in range(3, KB):
            nc.gpsimd.dma_start(W_bf[:, k, :], W_pkf[:, k, :])

        # Sync HWDGE: x3-x15 as f32 into a small staging ring; the ring
        # (and the bf16 ring the casts write) pace these loads behind
        # the PE's consumption so W and y keep their DMA share.
        x_stage = [None] * RB
        for rb in range(GROUP, RB):
            x_f32 = xfp.tile([P, D], mybir.dt.float32, name="x_f32", tag="x_f32")
            nc.sync.dma_start(x_f32[:], x[rb * P:(rb + 1) * P, :])
            x_stage[rb] = x_f32

        def cast_x(rb: int):
            x_bf = xpo.tile([P, D], mybir.dt.bfloat16, name="x_bfo", tag="x_bfo")
            nc.scalar.copy(x_bf[:], x_stage[rb][:])
            x_tiles[rb] = x_bf

        def warmup(n):
            for _ in range(n):
                nc.tensor.matmul(
                    warm_ps[:, 0:P], warm[:], warm[:, 0:1].to_broadcast([P, P]),
                    start=True, stop=True, skip_group_check=True,
                )

        def transpose(rb: int):
            # PE transposes the 8 k-tiles into one PSUM bank; DVE copies
            # them back to SBUF (scalar is busy with x casts + y issues).
            psT = psp.tile([P, KB, P], mybir.dt.bfloat16, name="psT", tag="psT", bufs=2)
            for k in range(KB):
                nc.tensor.transpose(psT[:, k, :], x_tiles[rb][:, k * P:(k + 1) * P], ident[:])
            xT = xtp.tile([P, KB, P], mybir.dt.bfloat16, name="xT", tag="xT")
            nc.vector.tensor_copy(xT[:], psT[:])
            return xT

        def evict(rb: int, pss) -> None:
            y_sb = yp.tile([P, F], mybir.dt.float32)
            for n in range(NB):
                nc.vector.tensor_add(
                    y_sb[:, n * NSPLIT:(n + 1) * NSPLIT],
                    pss[n][:],
                    b_bc[:, n * NSPLIT:(n + 1) * NSPLIT],
                )
            nc.scalar.dma_start(y[rb * P:(rb + 1) * P, :], y_sb[:])

        # PE warmup ramps the clock while the first x rowblock lands;
        # more warmups pad the gaps between the early transposes.
        warm_ps = ps0_tile()
        warmup(12)

        # Casts for the first back rowblocks so their transposes are
        # ready when phase 2 starts.
        cast_x(GROUP)
        cast_x(GROUP + 1)

        # Phase 1 - rowblocks 0..GROUP-1: transposes as x arrives, then
        # k-outer across 6 banks following W's k-block arrivals.
        xT_tiles = {}
        for r in range(GROUP):
            xT_tiles[r] = transpose(r)
            if r < GROUP - 1:
                warmup(4)
        psA = [(ps0_tile(), ps1_tile()) for _ in range(GROUP)]
        for k in range(KB):
            for r in range(GROUP):
                for n in range(NB):
                    nc.tensor.matmul(
                        psA[r][n][:],
                        xT_tiles[r][:, k, :],
                        W_bf[:, k, n * NSPLIT:(n + 1) * NSPLIT],
                        start=(k == 0),
                        stop=(k == KB - 1),
                    )
        for r in range(GROUP):
            evict(r, psA[r])

        # Phase 2 - rowblocks GROUP..RB-1 stream with k-inner; casts are
        # emitted two rowblocks ahead of their transposes.
        for rb in range(GROUP, RB):
            xT = transpose(rb)
            if rb + 2 < RB:
                cast_x(rb + 2)
            pss = (ps0_tile(), ps1_tile())
            for k in range(KB):
                for n in range(NB):
                    nc.tensor.matmul(
                        pss[n][:],
                        xT[:, k, :],
                        W_bf[:, k, n * NSPLIT:(n + 1) * NSPLIT],
                        start=(k == 0),
                        stop=(k == KB - 1),
                    )
            evict(rb, pss)

    nc.compile()
    return nc


_NC_CACHE: dict[int, bass.Bass] = {}


def _get_nc(rows: int = ROWS) -> bass.Bass:
    if rows not in _NC_CACHE:
        _NC_CACHE[rows] = build_nc(rows)
    return _NC_CACHE[rows]


def _run(in_maps, rows: int = ROWS, trace: bool = False):
    nc = _get_nc(rows)
    return run_bass_kernel_spmd(nc, in_maps, list(range(N_CORES)), trace=trace)


def kernel(x: np.ndarray, W: np.ndarray, b: np.ndarray) -> np.ndarray:
    x = np.ascontiguousarray(np.asarray(x, dtype=np.float32))
    W = np.ascontiguousarray(np.asarray(W, dtype=np.float32))
    b = np.ascontiguousarray(np.asarray(b, dtype=np.float32))
    x_flat = x.reshape(ROWS_TOTAL, D)
    in_maps = [
        {"x": np.ascontiguousarray(x_flat[c * ROWS:(c + 1) * ROWS]), "W": W, "b": b}
        for c in range(N_CORES)
    ]
    res = _run(in_maps, trace=bool(int(os.environ.get("BASS_KERNEL_TRACE", "0"))))
    y = np.concatenate([res.results[c]["y"] for c in range(N_CORES)], axis=0)
    return y.reshape(B, S, F)


# revision 13
# speedup vs baseline: 1.1496x; 1.0420x over previous
"""Trainium2 Bass kernel for HDGradientCompressionLayer forward.

Reference computation: y = einsum("bsd,df->bsf", x, W) + b
  x: (4, 4096, 1024) f32, W: (1024, 1024) f32, b: (1024,) f32.

Strategy (data-parallel across 8 cores, per sharding hint):
  Flatten x to (16384, 1024); each core gets 2048 rows. Per core the
  kernel computes y_shard = x_shard @ W + b:
    - rowblocks x0-x2 and all of W cast-load f32->bf16 on the SWDGE
      queue so W is fully resident by ~20us,
    - rowblocks x3-x15 load f32 on the sync HWDGE queue and are cast
      to bf16 by gpsimd; small staging rings pace these loads behind
      consumption so they never starve W or the y stores,
    - per rowblock the PE transposes the 8 [128,128] x tiles into PSUM
      (~0.6us burst), DVE evicts them to SBUF,
    - rowblocks 0-2 run k-outer across 6 PSUM banks so the PE can
      follow W's k-block-by-k-block arrival (this phase also absorbs
      the PE clock ramp; warmup matmuls fill the leading gaps),
    - rowblocks 3-15 then stream 16 bf16 matmuls each (N=512,
      PSUM-accumulated over the 8 d-blocks) at full PE rate,
    - DVE adds the (partition-broadcast) f32 bias during PSUM->SBUF
      eviction, scalar HWDGE stores f32 y rowblocks (4KB descriptors).
"""

import os
from contextlib import ExitStack

import numpy as np

import concourse.bass as bass
import concourse.bacc as bacc
import concourse.tile as tile
from concourse import mybir
from concourse.bass_utils import run_bass_kernel_spmd
from concourse.masks import make_identity

N_CORES = 8
B, S, D = 4, 4096, 1024
F = 1024
ROWS_TOTAL = B * S          # 16384
ROWS = ROWS_TOTAL // N_CORES  # 2048 per core
P = 128
NSPLIT = 512                # one PSUM bank of f32
KB = D // P                 # 8 contraction blocks
NB = F // NSPLIT            # 2 psum banks per rowblock
GROUP = 3                   # rowblocks in the k-outer warm phase


def build_nc(rows: int = ROWS) -> bass.Bass:
    nc = bacc.Bacc("TRN2", target_bir_lowering=False, debug=False)
    x = nc.dram_tensor("x", [rows, D], mybir.dt.float32, kind="ExternalInput").ap()
    W = nc.dram_tensor("W", [D, F], mybir.dt.float32, kind="ExternalInput").ap()
    b = nc.dram_tensor("b", [F], mybir.dt.float32, kind="ExternalInput").ap()
    y = nc.dram_tensor("y", [rows, F], mybir.dt.float32, kind="ExternalOutput").ap()

    RB = rows // P     # rowblocks

    with tile.TileContext(nc) as tc, ExitStack() as ctx:
        const = ctx.enter_context(tc.tile_pool(name="const", bufs=1))
        xpe = ctx.enter_context(tc.tile_pool(name="xpe", bufs=GROUP))
        xpo = ctx.enter_context(tc.tile_pool(name="xpo", bufs=2))
        xfp = ctx.enter_context(tc.tile_pool(name="xfp", bufs=2))
        xtp = ctx.enter_context(tc.tile_pool(name="xtp", bufs=RB))
        yp = ctx.enter_context(tc.tile_pool(name="yp", bufs=4))
        psp = ctx.enter_context(tc.tile_pool(name="psp", bufs=1, space="PSUM"))

        # W cast to bf16, laid out [p, k, f] with d = k*128 + p to match
        # the PE-transpose output layout of x.
        W_bf = const.tile([P, KB, F], mybir.dt.bfloat16)
        W_pkf = W.rearrange("(k p) f -> p k f", p=P)

        # Bias broadcast to all partitions, f32.
        b_bc = const.tile([P, F], mybir.dt.float32)

        # Identity for PE-based transposes; zeroed tile for clock warmup.
        ident = const.tile([P, P], mybir.dt.bfloat16)
        make_identity(nc, ident[:])
        warm = const.tile([P, P], mybir.dt.bfloat16)
        nc.any.memset(warm[:], 0.0)

        def ps0_tile():
            return psp.tile([P, NSPLIT], mybir.dt.float32, name="ps0", tag="ps0", bufs=GROUP)

        def ps1_tile():
            return psp.tile([P, NSPLIT], mybir.dt.float32, name="ps1", tag="ps1", bufs=GROUP)

        x_tiles = [None] * RB

        # SWDGE: x0-x2 cast-loads interleaved with W k-blocks and bias.
        def load_x_front(rb: int):
            x_bf = xpe.tile([P, D], mybir.dt.bfloat16, name="x_bf", tag="x_bf")
            nc.gpsimd.dma_start(x_bf[:], x[rb * P:(rb + 1) * P, :])  # cast load
            x_tiles[rb] = x_bf

        load_x_front(0)
        nc.gpsimd.dma_start(W_bf[:, 0, :], W_pkf[:, 0, :])
        load_x_front(1)
        nc.gpsimd.dma_start(W_bf[:, 1, :], W_pkf[:, 1, :])
        load_x_front(2)
        nc.gpsimd.dma_start(W_bf[:, 2, :], W_pkf[:, 2, :])
        for k in range(3, KB):
            nc.gpsimd.dma_start(W_bf[:, k, :], W_pkf[:, k, :])
        nc.gpsimd.dma_start(b_bc[:], b.rearrange("(o f) -> o f", o=1).to_broadcast([P, F]))

        # Sync HWDGE: x3-x15 as f32 into a small staging ring; the ring
        # (and the bf16 ring the casts write) pace these loads behind
        # the PE's consumption so W and y keep their DMA share.
        x_stage = [None] * RB
        for rb in range(GROUP, RB):
            x_f32 = xfp.tile([P, D], mybir.dt.float32, name="x_f32", tag="x_f32")
            nc.sync.dma_start(x_f32[:], x[rb * P:(rb + 1) * P, :])
            x_stage[rb] = x_f32

        def cast_x(rb: int):
            x_bf = xpo.tile([P, D], mybir.dt.bfloat16, name="x_bfo", tag="x_bfo")
            nc.gpsimd.tensor_copy(x_bf[:], x_stage[rb][:])
            x_tiles[rb] = x_bf

        def warmup(n):
            for _ in range(n):
                nc.tensor.matmul(
                    warm_ps[:, 0:P], warm[:], warm[:, 0:1].to_broadcast([P, P]),
                    start=True, stop=True, skip_group_check=True,
                )

        def transpose(rb: int):
            # PE transposes the 8 k-tiles into one PSUM bank; DVE and
            # scalar alternate copying them back to SBUF so the psT ring
            # recycles without queue-latency stalls.
            psT = psp.tile([P, KB, P], mybir.dt.bfloat16, name="psT", tag="psT", bufs=2)
            for k in range(KB):
                nc.tensor.transpose(psT[:, k, :], x_tiles[rb][:, k * P:(k + 1) * P], ident[:])
            xT = xtp.tile([P, KB, P], mybir.dt.bfloat16, name="xT", tag="xT")
            if rb % 2 == 0:
                nc.vector.tensor_copy(xT[:], psT[:])
            else:
                nc.scalar.copy(xT[:], psT[:])
            return xT

        def evict(rb: int, pss) -> None:
            y_sb = yp.tile([P, F], mybir.dt.float32)
            for n in range(NB):
                nc.vector.tensor_add(
                    y_sb[:, n * NSPLIT:(n + 1) * NSPLIT],
                    pss[n][:],
                    b_bc[:, n * NSPLIT:(n + 1) * NSPLIT],
                )
            nc.scalar.dma_start(y[rb * P:(rb + 1) * P, :], y_sb[:])

        # PE warmup ramps the clock while the first x rowblock lands;
        # more warmups pad the gaps between the early transposes.
        warm_ps = ps0_tile()
        warmup(12)

        # Casts for the first back rowblocks so their transposes are
        # ready when phase 2 starts.
        cast_x(GROUP)
        cast_x(GROUP + 1)

        # Phase 1 - rowblocks 0..GROUP-1: transposes as x arrives, then
        # k-outer across 6 banks following W's k-block arrivals.
        xT_tiles = {}
        for r in range(GROUP):
            xT_tiles[r] = transpose(r)
            if r < GROUP - 1:
                warmup(4)
        psA = [(ps0_tile(), ps1_tile()) for _ in range(GROUP)]
        for k in range(KB):
            for r in range(GROUP):
                for n in range(NB):
                    nc.tensor.matmul(
                        psA[r][n][:],
                        xT_tiles[r][:, k, :],
                        W_bf[:, k, n * NSPLIT:(n + 1) * NSPLIT],
                        start=(k == 0),
                        stop=(k == KB - 1),
                    )
        for r in range(GROUP):
            evict(r, psA[r])

        # Phase 2 - rowblocks GROUP..RB-1 stream with k-inner; casts are
        # emitted two rowblocks ahead of their transposes.
        for rb in range(GROUP, RB):
            xT = transpose(rb)
            if rb + 2 < RB:
                cast_x(rb + 2)
            pss = (ps0_tile(), ps1_tile())
            for k in range(KB):
                for n in range(NB):
                    nc.tensor.matmul(
                        pss[n][:],
                        xT[:, k, :],
                        W_bf[:, k, n * NSPLIT:(n + 1) * NSPLIT],
                        start=(k == 0),
                        stop=(k == KB - 1),
                    )
            evict(rb, pss)

    nc.compile()
    return nc


_NC_CACHE: dict[int, bass.Bass] = {}


def _get_nc(rows: int = ROWS) -> bass.Bass:
    if rows not in _NC_CACHE:
        _NC_CACHE[rows] = build_nc(rows)
    return _NC_CACHE[rows]


def _run(in_maps, rows: int = ROWS, trace: bool = False):
    nc = _get_nc(rows)
    return run_bass_kernel_spmd(nc, in_maps, list(range(N_CORES)), trace=trace)


def kernel(x: np.ndarray, W: np.ndarray, b: np.ndarray) -> np.ndarray:
    x = np.ascontiguousarray(np.asarray(x, dtype=np.float32))
    W = np.ascontiguousarray(np.asarray(W, dtype=np.float32))
    b = np.ascontiguousarray(np.asarray(b, dtype=np.float32))
    x_flat = x.reshape(ROWS_TOTAL, D)
    in_maps = [
        {"x": np.ascontiguousarray(x_flat[c * ROWS:(c + 1) * ROWS]), "W": W, "b": b}
        for c in range(N_CORES)
    ]
    res = _run(in_maps, trace=bool(int(os.environ.get("BASS_KERNEL_TRACE", "0"))))
    y = np.concatenate([res.results[c]["y"] for c in range(N_CORES)], axis=0)
    return y.reshape(B, S, F)


# revision 14
# speedup vs baseline: 1.1623x; 1.0110x over previous
"""Trainium2 Bass kernel for HDGradientCompressionLayer forward.

Reference computation: y = einsum("bsd,df->bsf", x, W) + b
  x: (4, 4096, 1024) f32, W: (1024, 1024) f32, b: (1024,) f32.

Strategy (data-parallel across 8 cores, per sharding hint):
  Flatten x to (16384, 1024); each core gets 2048 rows. Per core the
  kernel computes y_shard = x_shard @ W + b:
    - rowblocks x0-x2 and all of W cast-load f32->bf16 on the SWDGE
      queue so W is fully resident by ~20us,
    - rowblocks x3-x15 load f32 on the sync HWDGE queue and are cast
      to bf16 by DVE/scalar; small staging rings pace these loads
      behind consumption so they never starve W or the y stores,
    - per rowblock the PE transposes the 8 [128,128] x tiles into PSUM
      (~0.6us burst), DVE evicts them to SBUF,
    - rowblocks 0-2 run k-outer across 6 PSUM banks so the PE can
      follow W's k-block-by-k-block arrival (this phase also absorbs
      the PE clock ramp; warmup matmuls fill the leading gaps),
    - rowblocks 3-15 then stream 16 bf16 matmuls each (N=512,
      PSUM-accumulated over the 8 d-blocks) at full PE rate,
    - DVE adds the (partition-broadcast) f32 bias during PSUM->SBUF
      eviction, scalar HWDGE stores f32 y rowblocks (4KB descriptors).
"""

import os
from contextlib import ExitStack

import numpy as np

import concourse.bass as bass
import concourse.bacc as bacc
import concourse.tile as tile
from concourse import mybir
from concourse.bass_utils import run_bass_kernel_spmd
from concourse.masks import make_identity

N_CORES = 8
B, S, D = 4, 4096, 1024
F = 1024
ROWS_TOTAL = B * S          # 16384
ROWS = ROWS_TOTAL // N_CORES  # 2048 per core
P = 128
NSPLIT = 512                # one PSUM bank of f32
KB = D // P                 # 8 contraction blocks
NB = F // NSPLIT            # 2 psum banks per rowblock
GROUP = 3                   # rowblocks in the k-outer warm phase


def build_nc(rows: int = ROWS) -> bass.Bass:
    nc = bacc.Bacc("TRN2", target_bir_lowering=False, debug=False)
    x = nc.dram_tensor("x", [rows, D], mybir.dt.float32, kind="ExternalInput").ap()
    W = nc.dram_tensor("W", [D, F], mybir.dt.float32, kind="ExternalInput").ap()
    b = nc.dram_tensor("b", [F], mybir.dt.float32, kind="ExternalInput").ap()
    y = nc.dram_tensor("y", [rows, F], mybir.dt.float32, kind="ExternalOutput").ap()

    RB = rows // P     # rowblocks

    with tile.TileContext(nc) as tc, ExitStack() as ctx:
        const = ctx.enter_context(tc.tile_pool(name="const", bufs=1))
        xpe = ctx.enter_context(tc.tile_pool(name="xpe", bufs=GROUP))
        xpo = ctx.enter_context(tc.tile_pool(name="xpo", bufs=2))
        xfp = ctx.enter_context(tc.tile_pool(name="xfp", bufs=2))
        xtp = ctx.enter_context(tc.tile_pool(name="xtp", bufs=RB))
        yp = ctx.enter_context(tc.tile_pool(name="yp", bufs=4))
        psp = ctx.enter_context(tc.tile_pool(name="psp", bufs=1, space="PSUM"))

        # W cast to bf16, laid out [p, k, f] with d = k*128 + p to match
        # the PE-transpose output layout of x.
        W_bf = const.tile([P, KB, F], mybir.dt.bfloat16)
        W_pkf = W.rearrange("(k p) f -> p k f", p=P)

        # Bias broadcast to all partitions, f32.
        b_bc = const.tile([P, F], mybir.dt.float32)

        # Identity for PE-based transposes; zeroed tile for clock warmup.
        ident = const.tile([P, P], mybir.dt.bfloat16)
        make_identity(nc, ident[:])
        warm = const.tile([P, P], mybir.dt.bfloat16)
        nc.any.memset(warm[:], 0.0)

        def ps0_tile():
            return psp.tile([P, NSPLIT], mybir.dt.float32, name="ps0", tag="ps0", bufs=GROUP)

        def ps1_tile():
            return psp.tile([P, NSPLIT], mybir.dt.float32, name="ps1", tag="ps1", bufs=GROUP)

        x_tiles = [None] * RB

        # SWDGE: x0-x2 cast-loads interleaved with W k-blocks and bias.
        def load_x_front(rb: int):
            x_bf = xpe.tile([P, D], mybir.dt.bfloat16, name="x_bf", tag="x_bf")
            nc.gpsimd.dma_start(x_bf[:], x[rb * P:(rb + 1) * P, :])  # cast load
            x_tiles[rb] = x_bf

        load_x_front(0)
        nc.gpsimd.dma_start(W_bf[:, 0, :], W_pkf[:, 0, :])
        load_x_front(1)
        nc.gpsimd.dma_start(W_bf[:, 1, :], W_pkf[:, 1, :])
        load_x_front(2)
        nc.gpsimd.dma_start(W_bf[:, 2, :], W_pkf[:, 2, :])
        for k in range(3, KB):
            nc.gpsimd.dma_start(W_bf[:, k, :], W_pkf[:, k, :])
        nc.gpsimd.dma_start(b_bc[:], b.rearrange("(o f) -> o f", o=1).to_broadcast([P, F]))

        # Sync HWDGE: x3-x15 as f32 into a small staging ring; the ring
        # (and the bf16 ring the casts write) pace these loads behind
        # the PE's consumption so W and y keep their DMA share.
        x_stage = [None] * RB
        for rb in range(GROUP, RB):
            x_f32 = xfp.tile([P, D], mybir.dt.float32, name="x_f32", tag="x_f32")
            nc.sync.dma_start(x_f32[:], x[rb * P:(rb + 1) * P, :])
            x_stage[rb] = x_f32

        def cast_x(rb: int):
            x_bf = xpo.tile([P, D], mybir.dt.bfloat16, name="x_bfo", tag="x_bfo")
            if rb % 2 == 0:
                nc.vector.tensor_copy(x_bf[:], x_stage[rb][:])
            else:
                nc.scalar.copy(x_bf[:], x_stage[rb][:])
            x_tiles[rb] = x_bf

        def warmup(n):
            for _ in range(n):
                nc.tensor.matmul(
                    warm_ps[:, 0:P], warm[:], warm[:, 0:1].to_broadcast([P, P]),
                    start=True, stop=True, skip_group_check=True,
                )

        def transpose(rb: int):
            # PE transposes the 8 k-tiles into one PSUM bank; DVE and
            # scalar alternate copying them back to SBUF so the psT ring
            # recycles without queue-latency stalls.
            psT = psp.tile([P, KB, P], mybir.dt.bfloat16, name="psT", tag="psT", bufs=2)
            for k in range(KB):
                nc.tensor.transpose(psT[:, k, :], x_tiles[rb][:, k * P:(k + 1) * P], ident[:])
            xT = xtp.tile([P, KB, P], mybir.dt.bfloat16, name="xT", tag="xT")
            if rb % 2 == 0:
                nc.scalar.copy(xT[:], psT[:])
            else:
                nc.vector.tensor_copy(xT[:], psT[:])
            return xT

        def evict(rb: int, pss) -> None:
            y_sb = yp.tile([P, F], mybir.dt.float32)
            for n in range(NB):
                nc.vector.tensor_add(
                    y_sb[:, n * NSPLIT:(n + 1) * NSPLIT],
                    pss[n][:],
                    b_bc[:, n * NSPLIT:(n + 1) * NSPLIT],
                )
            nc.scalar.dma_start(y[rb * P:(rb + 1) * P, :], y_sb[:])

        # PE warmup ramps the clock while the first x rowblock lands;
        # more warmups pad the gaps between the early transposes.
        warm_ps = ps0_tile()
        warmup(12)

        # Casts for the first back rowblocks so their transposes are
        # ready when phase 2 starts.
        cast_x(GROUP)
        cast_x(GROUP + 1)

        # Phase 1 - rowblocks 0..GROUP-1: transposes as x arrives, then
        # k-outer across 6 banks following W's k-block arrivals.
        xT_tiles = {}
        for r in range(GROUP):
            xT_tiles[r] = transpose(r)
            if r < GROUP - 1:
                warmup(4)
        psA = [(ps0_tile(), ps1_tile()) for _ in range(GROUP)]
        for k in range(KB):
            for r in range(GROUP):
                for n in range(NB):
                    nc.tensor.matmul(
                        psA[r][n][:],
                        xT_tiles[r][:, k, :],
                        W_bf[:, k, n * NSPLIT:(n + 1) * NSPLIT],
                        start=(k == 0),
                        stop=(k == KB - 1),
                    )
        for r in range(GROUP):
            evict(r, psA[r])

        # Phase 2 - rowblocks GROUP..RB-1 stream with k-inner. The
        # transpose burst for rb+1 is emitted before rb's matmuls so the
        # copyback latency hides under the matmul stream; casts are
        # emitted two rowblocks ahead of their transposes.
        xT_next = transpose(GROUP)
        for rb in range(GROUP, RB):
            xT = xT_next
            if rb + 1 < RB:
                xT_next = transpose(rb + 1)
            if rb + 2 < RB:
                cast_x(rb + 2)
            pss = (ps0_tile(), ps1_tile())
            for k in range(KB):
                for n in range(NB):
                    nc.tensor.matmul(
                        pss[n][:],
                        xT[:, k, :],
                        W_bf[:, k, n * NSPLIT:(n + 1) * NSPLIT],
                        start=(k == 0),
                        stop=(k == KB - 1),
                    )
            evict(rb, pss)

    nc.compile()
    return nc


_NC_CACHE: dict[int, bass.Bass] = {}


def _get_nc(rows: int = ROWS) -> bass.Bass:
    if rows not in _NC_CACHE:
        _NC_CACHE[rows] = build_nc(rows)
    return _NC_CACHE[rows]


def _run(in_maps, rows: int = ROWS, trace: bool = False):
    nc = _get_nc(rows)
    return run_bass_kernel_spmd(nc, in_maps, list(range(N_CORES)), trace=trace)


def kernel(x: np.ndarray, W: np.ndarray, b: np.ndarray) -> np.ndarray:
    x = np.ascontiguousarray(np.asarray(x, dtype=np.float32))
    W = np.ascontiguousarray(np.asarray(W, dtype=np.float32))
    b = np.ascontiguousarray(np.asarray(b, dtype=np.float32))
    x_flat = x.reshape(ROWS_TOTAL, D)
    in_maps = [
        {"x": np.ascontiguousarray(x_flat[c * ROWS:(c + 1) * ROWS]), "W": W, "b": b}
        for c in range(N_CORES)
    ]
    res = _run(in_maps, trace=bool(int(os.environ.get("BASS_KERNEL_TRACE", "0"))))
    y = np.concatenate([res.results[c]["y"] for c in range(N_CORES)], axis=0)
    return y.reshape(B, S, F)


# revision 15
# speedup vs baseline: 1.2404x; 1.0672x over previous
"""Trainium2 Bass kernel for HDGradientCompressionLayer forward.

Reference computation: y = einsum("bsd,df->bsf", x, W) + b
  x: (4, 4096, 1024) f32, W: (1024, 1024) f32, b: (1024,) f32.

Strategy (data-parallel across 8 cores, per sharding hint):
  Flatten x to (16384, 1024); each core gets 2048 rows. Per core the
  kernel computes y_shard = x_shard @ W + b:
    - rowblocks x0-x2 and all of W cast-load f32->bf16 on the SWDGE
      queue so W is fully resident by ~20us,
    - rowblocks x3-x15 load f32 on the sync HWDGE queue and are cast
      to bf16 by DVE/scalar; small staging rings pace these loads
      behind consumption so they never starve W or the y stores,
    - per rowblock the PE transposes the 8 [128,128] x tiles into PSUM
      (~0.6us burst), DVE evicts them to SBUF,
    - rowblocks 0-2 run k-outer across 6 PSUM banks so the PE can
      follow W's k-block-by-k-block arrival (this phase also absorbs
      the PE clock ramp; warmup matmuls fill the leading gaps),
    - rowblocks 3-15 then stream 16 bf16 matmuls each (N=512,
      PSUM-accumulated over the 8 d-blocks) at full PE rate,
    - DVE adds the (partition-broadcast) f32 bias during PSUM->SBUF
      eviction, scalar HWDGE stores f32 y rowblocks (4KB descriptors).
"""

import os
from contextlib import ExitStack

import numpy as np

import concourse.bass as bass
import concourse.bacc as bacc
import concourse.tile as tile
from concourse import mybir
from concourse.bass_utils import run_bass_kernel_spmd
from concourse.masks import make_identity

N_CORES = 8
B, S, D = 4, 4096, 1024
F = 1024
ROWS_TOTAL = B * S          # 16384
ROWS = ROWS_TOTAL // N_CORES  # 2048 per core
P = 128
NSPLIT = 512                # one PSUM bank of f32
KB = D // P                 # 8 contraction blocks
NB = F // NSPLIT            # 2 psum banks per rowblock
GROUP = 3                   # rowblocks in the k-outer warm phase
XSYNC = 7                   # first rowblock fed through the sync f32 queue


def build_nc(rows: int = ROWS) -> bass.Bass:
    nc = bacc.Bacc("TRN2", target_bir_lowering=False, debug=False)
    x = nc.dram_tensor("x", [rows, D], mybir.dt.float32, kind="ExternalInput").ap()
    W = nc.dram_tensor("W", [D, F], mybir.dt.float32, kind="ExternalInput").ap()
    b = nc.dram_tensor("b", [F], mybir.dt.float32, kind="ExternalInput").ap()
    y = nc.dram_tensor("y", [rows, F], mybir.dt.float32, kind="ExternalOutput").ap()

    RB = rows // P     # rowblocks

    with tile.TileContext(nc) as tc, ExitStack() as ctx:
        const = ctx.enter_context(tc.tile_pool(name="const", bufs=1))
        xpe = ctx.enter_context(tc.tile_pool(name="xpe", bufs=7))
        xpo = ctx.enter_context(tc.tile_pool(name="xpo", bufs=2))
        xfp = ctx.enter_context(tc.tile_pool(name="xfp", bufs=2))
        xtp = ctx.enter_context(tc.tile_pool(name="xtp", bufs=RB))
        yp = ctx.enter_context(tc.tile_pool(name="yp", bufs=4))
        psp = ctx.enter_context(tc.tile_pool(name="psp", bufs=1, space="PSUM"))

        # W cast to bf16, laid out [p, k, f] with d = k*128 + p to match
        # the PE-transpose output layout of x.
        W_bf = const.tile([P, KB, F], mybir.dt.bfloat16)
        W_pkf = W.rearrange("(k p) f -> p k f", p=P)

        # Bias broadcast to all partitions, f32.
        b_bc = const.tile([P, F], mybir.dt.float32)

        # Identity for PE-based transposes; zeroed tile for clock warmup.
        ident = const.tile([P, P], mybir.dt.bfloat16)
        make_identity(nc, ident[:])
        warm = const.tile([P, P], mybir.dt.bfloat16)
        nc.vector.memset(warm[:], 0.0)

        def ps0_tile():
            return psp.tile([P, NSPLIT], mybir.dt.float32, name="ps0", tag="ps0", bufs=GROUP)

        def ps1_tile():
            return psp.tile([P, NSPLIT], mybir.dt.float32, name="ps1", tag="ps1", bufs=GROUP)

        x_tiles = [None] * RB

        # SWDGE: x0-x2 cast-loads interleaved with W k-blocks and bias.
        def load_x_front(rb: int):
            x_bf = xpe.tile([P, D], mybir.dt.bfloat16, name="x_bf", tag="x_bf")
            nc.gpsimd.dma_start(x_bf[:], x[rb * P:(rb + 1) * P, :])  # cast load
            x_tiles[rb] = x_bf

        load_x_front(0)
        nc.gpsimd.dma_start(W_bf[:, 0, :], W_pkf[:, 0, :])
        load_x_front(1)
        nc.gpsimd.dma_start(W_bf[:, 1, :], W_pkf[:, 1, :])
        load_x_front(2)
        nc.gpsimd.dma_start(W_bf[:, 2, :], W_pkf[:, 2, :])
        for k in range(3, KB):
            nc.gpsimd.dma_start(W_bf[:, k, :], W_pkf[:, k, :])
        nc.gpsimd.dma_start(b_bc[:], b.rearrange("(o f) -> o f", o=1).to_broadcast([P, F]))
        for rb in range(GROUP, XSYNC):
            load_x_front(rb)

        # Sync HWDGE: x7-x15 as f32 into a small staging ring; the ring
        # (and the bf16 ring the casts write) pace these loads behind
        # the PE's consumption so W and y keep their DMA share.
        x_stage = [None] * RB
        for rb in range(XSYNC, RB):
            x_f32 = xfp.tile([P, D], mybir.dt.float32, name="x_f32", tag="x_f32")
            nc.sync.dma_start(x_f32[:], x[rb * P:(rb + 1) * P, :])
            x_stage[rb] = x_f32

        def cast_x(rb: int):
            x_bf = xpo.tile([P, D], mybir.dt.bfloat16, name="x_bfo", tag="x_bfo")
            if rb % 2 == 0:
                nc.vector.tensor_copy(x_bf[:], x_stage[rb][:])
            else:
                nc.scalar.copy(x_bf[:], x_stage[rb][:])
            x_tiles[rb] = x_bf

        def warmup(n):
            for _ in range(n):
                nc.tensor.matmul(
                    warm_ps[:, 0:P], warm[:], warm[:, 0:1].to_broadcast([P, P]),
                    start=True, stop=True, skip_group_check=True,
                )

        def transpose(rb: int):
            # PE transposes the 8 k-tiles into one PSUM bank; DVE and
            # scalar alternate copying them back to SBUF so the psT ring
            # recycles without queue-latency stalls.
            psT = psp.tile([P, KB, P], mybir.dt.bfloat16, name="psT", tag="psT", bufs=2)
            for k in range(KB):
                nc.tensor.transpose(psT[:, k, :], x_tiles[rb][:, k * P:(k + 1) * P], ident[:])
            xT = xtp.tile([P, KB, P], mybir.dt.bfloat16, name="xT", tag="xT")
            if rb % 2 == 0:
                nc.scalar.copy(xT[:], psT[:])
            else:
                nc.vector.tensor_copy(xT[:], psT[:])
            return xT

        def evict(rb: int, pss) -> None:
            y_sb = yp.tile([P, F], mybir.dt.float32)
            for n in range(NB):
                nc.vector.tensor_add(
                    y_sb[:, n * NSPLIT:(n + 1) * NSPLIT],
                    pss[n][:],
                    b_bc[:, n * NSPLIT:(n + 1) * NSPLIT],
                )
            nc.scalar.dma_start(y[rb * P:(rb + 1) * P, :], y_sb[:])

        # PE warmup ramps the clock while the first x rowblock lands;
        # more warmups pad the gaps between the early transposes.
        warm_ps = ps0_tile()
        warmup(12)

        # Casts for the first back rowblocks so their transposes are
        # ready when the matmul stream reaches them.
        cast_x(XSYNC)
        cast_x(XSYNC + 1)

        # Phase 1 - rowblocks 0..GROUP-1: transposes as x arrives, then
        # k-outer across 6 banks following W's k-block arrivals.
        xT_tiles = {}
        for r in range(GROUP):
            xT_tiles[r] = transpose(r)
            if r < GROUP - 1:
                warmup(4)
        psA = [(ps0_tile(), ps1_tile()) for _ in range(GROUP)]
        for k in range(KB):
            for r in range(GROUP):
                for n in range(NB):
                    nc.tensor.matmul(
                        psA[r][n][:],
                        xT_tiles[r][:, k, :],
                        W_bf[:, k, n * NSPLIT:(n + 1) * NSPLIT],
                        start=(k == 0),
                        stop=(k == KB - 1),
                    )
        for r in range(GROUP):
            evict(r, psA[r])

        # Phase 2 - rowblocks GROUP..RB-1 stream with k-inner. The
        # transpose burst for rb+1 is emitted before rb's matmuls so the
        # copyback latency hides under the matmul stream; casts are
        # emitted two rowblocks ahead of their transposes.
        xT_next = transpose(GROUP)
        for rb in range(GROUP, RB):
            xT = xT_next
            if rb + 1 < RB:
                xT_next = transpose(rb + 1)
            if XSYNC + 2 <= rb + 2 < RB:
                cast_x(rb + 2)
            pss = (ps0_tile(), ps1_tile())
            for k in range(KB):
                for n in range(NB):
                    nc.tensor.matmul(
                        pss[n][:],
                        xT[:, k, :],
                        W_bf[:, k, n * NSPLIT:(n + 1) * NSPLIT],
                        start=(k == 0),
                        stop=(k == KB - 1),
                    )
            evict(rb, pss)

    nc.compile()
    return nc


_NC_CACHE: dict[int, bass.Bass] = {}


def _get_nc(rows: int = ROWS) -> bass.Bass:
    if rows not in _NC_CACHE:
        _NC_CACHE[rows] = build_nc(rows)
    return _NC_CACHE[rows]


def _run(in_maps, rows: int = ROWS, trace: bool = False):
    nc = _get_nc(rows)
    return run_bass_kernel_spmd(nc, in_maps, list(range(N_CORES)), trace=trace)


def kernel(x: np.ndarray, W: np.ndarray, b: np.ndarray) -> np.ndarray:
    x = np.ascontiguousarray(np.asarray(x, dtype=np.float32))
    W = np.ascontiguousarray(np.asarray(W, dtype=np.float32))
    b = np.ascontiguousarray(np.asarray(b, dtype=np.float32))
    x_flat = x.reshape(ROWS_TOTAL, D)
    in_maps = [
        {"x": np.ascontiguousarray(x_flat[c * ROWS:(c + 1) * ROWS]), "W": W, "b": b}
        for c in range(N_CORES)
    ]
    res = _run(in_maps, trace=bool(int(os.environ.get("BASS_KERNEL_TRACE", "0"))))
    y = np.concatenate([res.results[c]["y"] for c in range(N_CORES)], axis=0)
    return y.reshape(B, S, F)
